# revision 1
# baseline (speedup 1.0000x reference)
"""Masked dot-product attention (B=16, Lq=Lk=2048, D=Dv=256, fp32) on 8 trn2 cores.

Work-flattened design: the computation is a list of independent units
(batch b, 512-query slot, 128-key block). Units per batch = ceil(valid_len/128)
x 4 query slots; the host splits each (b, q-slot) item into chunks of <= SMAX
key blocks, sorts chunks by size, and deals them round-robin into a static
per-core slot template (caps = per-tier max chunk length, identical on every
core -> one SPMD program, no runtime gating). Pad units have zeroed K/V' so
they contribute nothing. The mask also lives in V' (masked key rows zeroed,
including the appended ones-column), so the exp needs no per-key bias and two
units share one ACT instruction ([128,1024] across 2 PSUM banks).

Per unit: S^T = (K^T blk).T @ (Q^T slot) (2 matmuls N=512, contract d)
          P^T = exp(S^T / 16)             (one ACT per 2 units)
          po[qs] += (P^T slice).T @ V'    (4 matmuls N=257, accumulate over
                                           the chunk; col 256 = sum exp)
Slot end: copy po -> SBUF (bf16) and DMA out; host combines chunk partials
(numerator/denominator sums) and normalizes.
"""

import contextlib

import numpy as np
import ml_dtypes

import concourse.bass as bass
import concourse.bacc as bacc
import concourse.tile as tile
from concourse import mybir
from concourse.bass_utils import run_bass_kernel_spmd

B, LQ, LK, D, DV = 16, 2048, 2048, 256, 256
NCORES = 8
KB = 128               # keys per unit
QT = 512               # queries per slot
NQS = QT // 128        # 4 query sub-tiles per slot
NDC = D // 128         # 2 contraction chunks
SMAX = 8               # max key-blocks per chunk (slot fill)

BF16 = mybir.dt.bfloat16
F32 = mybir.dt.float32
EXP = mybir.ActivationFunctionType.Exp

_progs = {}

PH = 1  # slots per load phase


def phase_layout(caps):
    """Group slots into load phases; flat per-partition element offsets.

    Per phase k (slots ph, pc units): [qt: NDC*nph*QT][kt: NDC*pc*KB][vp: pc*(DV+1)]
    Returns (phases, base[k], kt_off[k], vp_off[k], total).
    """
    nslot = len(caps)
    phases = [list(range(k, min(k + PH, nslot))) for k in range(0, nslot, PH)]
    base, kt_off, vp_off = [], [], []
    tot = 0
    for ph in phases:
        nph = len(ph)
        pc = sum(caps[j] for j in ph)
        base.append(tot)
        kt_off.append(NDC * nph * QT)
        vp_off.append(NDC * nph * QT + NDC * pc * KB)
        tot += NDC * nph * QT + NDC * pc * KB + pc * (DV + 1)
    return phases, base, kt_off, vp_off, tot


def _build_program(
    caps,
    timing_loop=False,
    ps_bufs=2,
    pt_bufs=3,
    ob_bufs=6,
    skip_loads=False,
    skip_out=False,
    out_ring="scalar",
    split_loads=1,
    copy_split=True,
    skip_compute=False,
    load_rings=1,
):
    """caps: tuple of per-slot unit capacities (same on all cores)."""
    nslot = len(caps)
    cap = sum(caps)
    nc = bacc.Bacc(
        "TRN2",
        target_bir_lowering=False,
        debug=False,
        num_devices=NCORES,
        enable_asserts=False,
    )
    phases, pbase, pkt_off, pvp_off, ptot = phase_layout(caps)
    inp_d = nc.dram_tensor("inp", [128, ptot], BF16, kind="ExternalInput").ap()
    if timing_loop:
        ri_d = nc.dram_tensor("riter", [1, 1], mybir.dt.int32, kind="ExternalInput").ap()
    out_d = nc.dram_tensor(
        "out", [128, nslot * NQS * (DV + 1)], BF16, kind="ExternalOutput"
    ).ap()

    # flattened unit list: (slot j, t within slot, first, last)
    units = []
    for j, cj in enumerate(caps):
        for t in range(cj):
            units.append((j, t, t == 0, t == cj - 1))

    ubase = [sum(caps[:j]) for j in range(nslot)]

    with tile.TileContext(nc) as tc:
        with (
            tc.tile_pool(name="inp", bufs=1) as inp,
            tc.tile_pool(name="work", bufs=3) as work,
            tc.tile_pool(name="outp", bufs=2) as outp,
            tc.tile_pool(name="psum", bufs=1, space="PSUM") as psum,
            contextlib.ExitStack() as body_cm,
        ):
            if timing_loop:
                ri_sb = work.tile([1, 1], mybir.dt.int32, tag="ri", bufs=1)
                nc.sync.dma_start(ri_sb, ri_d)
                riter = nc.values_load(
                    ri_sb, min_val=1, max_val=1 << 20, skip_runtime_bounds_check=True
                )
            # Preload the exp table set (~2.7us) before the main body.
            warm_in = work.tile([128, 1], F32, tag="warm", bufs=1)
            warm_out = work.tile([128, 1], F32, tag="warm2", bufs=1)
            nc.vector.memset(warm_in, 0.0)
            nc.scalar.activation(warm_out, warm_in, EXP, bias=warm_in, scale=1.0)

            # Input tiles: one flat contiguous [128, L] tile per load phase
            # (PH slots) -> a single max-bandwidth DMA per phase; the phase's
            # next-iteration (timing loop) load is re-issued right after its
            # compute, overlapping the rest of the iteration.
            ph_of = {}
            ph_pc = []
            for k, ph in enumerate(phases):
                for j in ph:
                    ph_of[j] = (k, ph.index(j), sum(caps[i] for i in ph[: ph.index(j)]))
                ph_pc.append(sum(caps[j] for j in ph))
            ph_tiles = []
            for k, ph in enumerate(phases):
                L = (pbase[k + 1] if k + 1 < len(phases) else ptot) - pbase[k]
                ph_tiles.append(inp.tile([128, L], BF16, tag=f"ph{k}", name=f"ph{k}"))
            if skip_loads:
                for k in range(len(phases)):
                    nc.vector.memset(ph_tiles[k], 0.01)

            def kt_ap(j, t, c):
                k, _, uoff = ph_of[j]
                off = pkt_off[k] + c * ph_pc[k] * KB + (uoff + t) * KB
                return ph_tiles[k][:, off : off + KB]

            def qt_ap(j, c):
                k, joff, _ = ph_of[j]
                off = c * len(phases[k]) * QT + joff * QT
                return ph_tiles[k][:, off : off + QT]

            def vp_ap(j, t):
                k, _, uoff = ph_of[j]
                off = pvp_off[k] + (uoff + t) * (DV + 1)
                return ph_tiles[k][:, off : off + (DV + 1)]

            def load_phase(k):
                L = (pbase[k + 1] if k + 1 < len(phases) else ptot) - pbase[k]
                eng = nc.sync if (load_rings == 1 or k % 2 == 0) else nc.scalar
                eng.dma_start(ph_tiles[k], inp_d[:, pbase[k] : pbase[k] + L])

            if not skip_loads:
                # Prologue load of every phase. In the timing loop, each
                # phase's load is RE-issued inside the body right after that
                # phase's compute (data for the next iteration) so DMA spreads
                # across the whole iteration and never stalls compute.
                for k in range(len(phases)):
                    load_phase(k)

            if timing_loop:
                body_cm.enter_context(tc.For_i(0, riter))

            po = None
            out_engine = {"scalar": nc.scalar, "sync": nc.sync, "gpsimd": nc.gpsimd}[
                out_ring
            ]

            def open_po():
                return [
                    psum.tile([128, DV + 1], F32, tag=f"po{qs}", bufs=1, name=f"po{qs}")
                    for qs in range(NQS)
                ]

            def close_po(j):
                if skip_out:
                    return
                W = NQS * (DV + 1)
                ob = outp.tile([128, W], BF16, tag="ob", bufs=ob_bufs, name="ob")
                for qs in range(NQS):
                    dst = ob[:, qs * (DV + 1) : (qs + 1) * (DV + 1)]
                    if copy_split and qs >= 2:
                        nc.scalar.copy(dst, po[qs])
                    else:
                        nc.vector.tensor_copy(dst, po[qs])
                out_engine.dma_start(out_d[:, j * W : (j + 1) * W], ob)

            # Units are processed in pairs sharing one ACT instruction.
            # Software-pipelined emission: pair p+1's S matmuls are emitted
            # before pair p's PV matmuls so the PE never waits on the exp.
            pairs = []
            i = 0
            while i < len(units):
                pairs.append(units[i : i + 2])
                i += 2
            npairs = len(pairs)

            def emit_s(p):
                ps = psum.tile([128, 1024], F32, tag="ps", bufs=ps_bufs, name="ps")
                for pi, (j, t, first, last) in enumerate(pairs[p]):
                    for c in range(NDC):
                        nc.tensor.matmul(
                            ps[:, pi * 512 : (pi + 1) * 512],
                            kt_ap(j, t, c),
                            qt_ap(j, c),
                            start=(c == 0),
                            stop=(c == NDC - 1),
                        )
                return ps

            def emit_exp(p, ps):
                npair = len(pairs[p])
                pt = work.tile(
                    [128, npair * 512], BF16, tag=f"pt{npair}", bufs=pt_bufs, name="pt"
                )
                nc.scalar.activation(pt, ps[:, 0 : npair * 512], EXP, scale=0.0625)
                return pt

            def emit_pv(p, pt):
                nonlocal po
                for pi, (j, t, first, last) in enumerate(pairs[p]):
                    if first:
                        po = open_po()
                    vp_a = vp_ap(j, t)
                    for qs in range(NQS):
                        nc.tensor.matmul(
                            po[qs],
                            pt[:, pi * 512 + qs * 128 : pi * 512 + (qs + 1) * 128],
                            vp_a,
                            start=first,
                            stop=last,
                        )
                    if last:
                        close_po(j)
                        k = ph_of[j][0]
                        if (
                            timing_loop
                            and not skip_loads
                            and j == phases[k][-1]
                        ):
                            # reload this phase for the next loop iteration
                            load_phase(k)

            if skip_compute:
                pairs = []
                if not skip_loads:
                    for k in range(len(phases)):
                        load_phase(k)
            if pairs:
                ps_p = emit_s(0)
                pt_p = emit_exp(0, ps_p)
            for p in range(npairs):
                if skip_compute:
                    break
                ps_n = emit_s(p + 1) if p + 1 < npairs else None
                emit_pv(p, pt_p)
                if ps_n is not None:
                    pt_p = emit_exp(p + 1, ps_n)

    nc.compile()
    return nc


def get_program(caps, timing_loop=False, **opts):
    key = (tuple(caps), bool(timing_loop), tuple(sorted(opts.items())))
    if key not in _progs:
        _progs[key] = _build_program(tuple(caps), timing_loop=timing_loop, **opts)
    return _progs[key]


def plan(valid_len):
    """Split work into chunks and deal them into a static slot template.

    Returns (caps, assign) where caps[j] = units in slot j (same all cores)
    and assign[c][j] = (b, qi, k0, klen) or None.
    """
    vl = np.asarray(valid_len).astype(np.int64)
    nkb = [max(1, -(-int(v) // KB)) for v in vl]
    chunks = []
    for b in range(B):
        for qi in range(LQ // QT):
            n, k0 = nkb[b], 0
            while n > 0:
                l = min(SMAX, n)
                chunks.append((b, qi, k0, l))
                k0 += l
                n -= l
    chunks.sort(key=lambda ch: -ch[3])
    nslot = -(-len(chunks) // NCORES)
    caps = tuple(chunks[j * NCORES][3] for j in range(nslot))
    assign = [[None] * nslot for _ in range(NCORES)]
    for idx, ch in enumerate(chunks):
        j, c = divmod(idx, NCORES)
        assign[c][j] = ch
    return caps, assign


def pack_core_inputs(query, key, value, valid_len, caps, assign_c):
    """Pack one core's inputs into the flat phase-major layout."""
    bf16 = ml_dtypes.bfloat16
    phases, pbase, pkt_off, pvp_off, ptot = phase_layout(caps)
    vl = np.asarray(valid_len).astype(np.int64)
    flat = np.zeros((128, ptot), dtype=bf16)
    for k, ph in enumerate(phases):
        nph = len(ph)
        pc = sum(caps[j] for j in ph)
        qtS = np.zeros((128, NDC, nph * QT), dtype=bf16)
        ktP = np.zeros((128, NDC, pc * KB), dtype=bf16)
        vpP = np.zeros((128, pc, DV + 1), dtype=bf16)
        u0 = 0
        for joff, j in enumerate(ph):
            ch = assign_c[j]
            cj = caps[j]
            if ch is not None:
                b, qi, k0, klen = ch
                qs = query[b, qi * QT : (qi + 1) * QT, :].T.reshape(NDC, 128, QT)
                qtS[:, :, joff * QT : (joff + 1) * QT] = qs.transpose(1, 0, 2).astype(
                    bf16
                )
                kbeg, kend = k0 * KB, (k0 + klen) * KB
                ks = key[b, kbeg:kend, :].T.reshape(NDC, 128, klen * KB)
                ktP[:, :, u0 * KB : (u0 + klen) * KB] = ks.transpose(1, 0, 2).astype(
                    bf16
                )
                vv = np.concatenate(
                    [value[b, kbeg:kend], np.ones((klen * KB, 1), np.float32)], axis=1
                )
                vv[vl[b] - kbeg :] = 0.0  # mask: zero rows for k >= valid_len
                vpP[:, u0 : u0 + klen, :] = (
                    vv.reshape(klen, KB, DV + 1).transpose(1, 0, 2).astype(bf16)
                )
            u0 += cj
        L = (pbase[k + 1] if k + 1 < len(phases) else ptot) - pbase[k]
        blk = np.concatenate(
            [qtS.reshape(128, -1), ktP.reshape(128, -1), vpP.reshape(128, -1)], axis=1
        )
        assert blk.shape[1] == L
        flat[:, pbase[k] : pbase[k] + L] = blk
    return {"inp": flat}


def kernel(query, key, value, valid_len, _res_out=None):
    query = np.asarray(query, dtype=np.float32)
    key = np.asarray(key, dtype=np.float32)
    value = np.asarray(value, dtype=np.float32)

    caps, assign = plan(valid_len)
    in_maps = [
        pack_core_inputs(query, key, value, valid_len, caps, assign[c])
        for c in range(NCORES)
    ]
    nc = get_program(caps)
    res = run_bass_kernel_spmd(nc, in_maps, core_ids=list(range(NCORES)))
    if _res_out is not None:
        _res_out.append(res)

    nslot = len(caps)
    num = np.zeros((B, LQ // QT, QT, DV), dtype=np.float32)
    den = np.zeros((B, LQ // QT, QT), dtype=np.float32)
    for c in range(NCORES):
        # out layout: [128, nslot*NQS*(DV+1)] -> [128, nslot, NQS, DV+1]
        r = (
            np.asarray(res.results[c]["out"])
            .astype(np.float32)
            .reshape(128, nslot, NQS, DV + 1)
        )
        for j, ch in enumerate(assign[c]):
            if ch is None:
                continue
            b, qi, k0, klen = ch
            blk = r[:, j].transpose(1, 0, 2).reshape(QT, DV + 1)  # q-major
            num[b, qi] += blk[:, :DV]
            den[b, qi] += blk[:, DV]
    out = num / den[..., None]
    return out.reshape(B, LQ, DV).astype(np.float32)



# revision 21
# speedup vs baseline: 1.1670x; 1.1670x over previous
"""Masked dot-product attention (B=16, Lq=Lk=2048, D=Dv=256, fp32) on 8 trn2 cores.

Work-flattened design: the computation is a list of independent units
(batch b, 512-query slot, 128-key block). Units per batch = ceil(valid_len/128)
x 4 query slots; the host splits each (b, q-slot) item into chunks of <= SMAX
key blocks, sorts chunks by size, and deals them round-robin into a static
per-core slot template (caps = per-tier max chunk length, identical on every
core -> one SPMD program, no runtime gating). Pad units have zeroed K/V' so
they contribute nothing. The mask also lives in V' (masked key rows zeroed,
including the appended ones-column), so the exp needs no per-key bias and two
units share one ACT instruction ([128,1024] across 2 PSUM banks).

Per unit: S^T = (K^T blk).T @ (Q^T slot) (2 matmuls N=512, contract d)
          P^T = exp(S^T / 16)             (one ACT per 2 units)
          po[qs] += (P^T slice).T @ V'    (4 matmuls N=257, accumulate over
                                           the chunk; col 256 = sum exp)
Slot end: copy po -> SBUF (bf16) and DMA out; host combines chunk partials
(numerator/denominator sums) and normalizes.
"""

import contextlib

import numpy as np
import ml_dtypes

import concourse.bass as bass
import concourse.bacc as bacc
import concourse.tile as tile
from concourse import mybir
from concourse.bass_utils import run_bass_kernel_spmd

B, LQ, LK, D, DV = 16, 2048, 2048, 256, 256
NCORES = 8
KB = 128               # keys per unit
QT = 512               # queries per slot
NQS = QT // 128        # 4 query sub-tiles per slot
NDC = D // 128         # 2 contraction chunks
SMAX = 8               # max key-blocks per chunk (slot fill)

BF16 = mybir.dt.bfloat16
F32 = mybir.dt.float32
EXP = mybir.ActivationFunctionType.Exp

_progs = {}

PH = 2  # slots per load phase


def phase_layout(caps):
    """Group slots into load phases; flat per-partition element offsets.

    Per phase k (slots ph, pc units): [qt: NDC*nph*QT][kt: NDC*pc*KB][vp: pc*(DV+1)]
    Returns (phases, base[k], kt_off[k], vp_off[k], total).
    """
    nslot = len(caps)
    phases = [list(range(k, min(k + PH, nslot))) for k in range(0, nslot, PH)]
    base, kt_off, vp_off = [], [], []
    tot = 0
    for ph in phases:
        nph = len(ph)
        pc = sum(caps[j] for j in ph)
        base.append(tot)
        kt_off.append(NDC * nph * QT)
        vp_off.append(NDC * nph * QT + NDC * pc * KB)
        tot += NDC * nph * QT + NDC * pc * KB + pc * (DV + 1)
    return phases, base, kt_off, vp_off, tot


def _build_program(
    caps,
    timing_loop=False,
    ps_bufs=4,
    pt_bufs=3,
    ob_bufs=6,
    skip_loads=False,
    skip_out=False,
    out_ring="scalar",
    split_loads=1,
    copy_split=True,
    skip_compute=False,
    load_rings=1,
    unit_act=True,
    depth=2,
    copy_pattern="vvvv",
    decouple_loads=False,
    preload=False,
    skip_out_dma=False,
    out_batch=1,
    fuse_passes=True,
    load_frac=1.0,
    stag=True,
):
    """caps: tuple of per-slot unit capacities (same on all cores)."""
    nslot = len(caps)
    cap = sum(caps)
    nc = bacc.Bacc(
        "TRN2",
        target_bir_lowering=False,
        debug=False,
        num_devices=NCORES,
        enable_asserts=False,
    )
    phases, pbase, pkt_off, pvp_off, ptot = phase_layout(caps)
    inp_d = nc.dram_tensor("inp", [128, ptot], BF16, kind="ExternalInput").ap()
    if timing_loop:
        ri_d = nc.dram_tensor("riter", [1, 1], mybir.dt.int32, kind="ExternalInput").ap()
    out_d = nc.dram_tensor(
        "out", [128, nslot * NQS * (DV + 1)], BF16, kind="ExternalOutput"
    ).ap()

    # flattened unit list: (slot j, t within slot, first, last)
    units = []
    for j, cj in enumerate(caps):
        for t in range(cj):
            units.append((j, t, t == 0, t == cj - 1))

    ubase = [sum(caps[:j]) for j in range(nslot)]

    with tile.TileContext(nc) as tc:
        with (
            tc.tile_pool(name="inp", bufs=1) as inp,
            tc.tile_pool(name="work", bufs=3) as work,
            tc.tile_pool(name="outp", bufs=2) as outp,
            tc.tile_pool(name="psum", bufs=1, space="PSUM") as psum,
            contextlib.ExitStack() as body_cm,
        ):
            if timing_loop:
                ri_sb = work.tile([1, 1], mybir.dt.int32, tag="ri", bufs=1)
                nc.sync.dma_start(ri_sb, ri_d)
                riter = nc.values_load(
                    ri_sb, min_val=1, max_val=1 << 20, skip_runtime_bounds_check=True
                )
            # Preload the exp table set (~2.7us) before the main body.
            warm_in = work.tile([128, 1], F32, tag="warm", bufs=1)
            warm_out = work.tile([128, 1], F32, tag="warm2", bufs=1)
            nc.vector.memset(warm_in, 0.0)
            nc.scalar.activation(warm_out, warm_in, EXP, bias=warm_in, scale=1.0)

            # Input tiles: one flat contiguous [128, L] tile per load phase
            # (PH slots) -> a single max-bandwidth DMA per phase. In the
            # timing loop the body is unrolled into TWO passes with
            # double-buffered phase tiles: pass p computes from buffer p while
            # its per-phase reloads fill buffer 1-p for the next pass, so
            # loads never gate compute.
            ph_of = {}
            ph_pc = []
            for k, ph in enumerate(phases):
                for j in ph:
                    ph_of[j] = (k, ph.index(j), sum(caps[i] for i in ph[: ph.index(j)]))
                ph_pc.append(sum(caps[j] for j in ph))
            n_pb = 2 if (timing_loop and not skip_loads) else 1
            ph_tiles_pb = []
            for pb in range(n_pb):
                tiles = []
                for k, ph in enumerate(phases):
                    L = (pbase[k + 1] if k + 1 < len(phases) else ptot) - pbase[k]
                    tiles.append(
                        inp.tile([128, L], BF16, tag=f"ph{k}p{pb}", name=f"ph{k}p{pb}")
                    )
                ph_tiles_pb.append(tiles)
            ph_tiles = ph_tiles_pb[0]
            cur_pb = [0]
            if skip_loads:
                for k in range(len(phases)):
                    nc.vector.memset(ph_tiles[k], 0.01)

            if decouple_loads:
                dummy = inp.tile([128, 1024], BF16, tag="dummy", bufs=1, name="dummy")
                nc.vector.memset(dummy, 0.01)

                def kt_ap(j, t, c):
                    return dummy[:, 0:KB]

                def qt_ap(j, c):
                    return dummy[:, 0:QT]

                def vp_ap(j, t):
                    return dummy[:, 0 : DV + 1]
            else:

                def kt_ap(j, t, c):
                    k, _, uoff = ph_of[j]
                    off = pkt_off[k] + c * ph_pc[k] * KB + (uoff + t) * KB
                    return ph_tiles_pb[cur_pb[0]][k][:, off : off + KB]

                def qt_ap(j, c):
                    k, joff, _ = ph_of[j]
                    off = c * len(phases[k]) * QT + joff * QT
                    return ph_tiles_pb[cur_pb[0]][k][:, off : off + QT]

                def vp_ap(j, t):
                    k, _, uoff = ph_of[j]
                    off = pvp_off[k] + (uoff + t) * (DV + 1)
                    return ph_tiles_pb[cur_pb[0]][k][:, off : off + (DV + 1)]

            def load_phase(k, pb=0):
                L = (pbase[k + 1] if k + 1 < len(phases) else ptot) - pbase[k]
                Lf = max(1, int(L * load_frac))
                eng = nc.sync if (load_rings == 1 or k % 2 == 0) else nc.scalar
                eng.dma_start(
                    ph_tiles_pb[pb][k][:, 0:Lf], inp_d[:, pbase[k] : pbase[k] + Lf]
                )

            if not skip_loads:
                # Prologue load of every phase. In the timing loop, each
                # phase's load is RE-issued inside the body right after that
                # phase's compute (data for the next iteration) so DMA spreads
                # across the whole iteration and never stalls compute.
                for k in range(len(phases)):
                    load_phase(k)

            if timing_loop:
                body_cm.enter_context(
                    tc.For_i(0, riter, staggered_reset=stag)
                )

            po = None
            out_engine = {"scalar": nc.scalar, "sync": nc.sync, "gpsimd": nc.gpsimd}[
                out_ring
            ]

            def open_po():
                return [
                    psum.tile([128, DV + 1], F32, tag=f"po{qs}", bufs=1, name=f"po{qs}")
                    for qs in range(NQS)
                ]

            ob_state = {"ob": None, "j0": -1}

            def close_po(j):
                if skip_out:
                    return
                W = NQS * (DV + 1)
                jb = j % out_batch
                if jb == 0:
                    ob_state["ob"] = outp.tile(
                        [128, W * out_batch], BF16, tag="ob", bufs=ob_bufs, name="ob"
                    )
                    ob_state["j0"] = j
                ob = ob_state["ob"]
                for qs in range(NQS):
                    dst = ob[:, jb * W + qs * (DV + 1) : jb * W + (qs + 1) * (DV + 1)]
                    if unit_act:
                        if copy_pattern[qs] == "s":
                            nc.scalar.copy(dst, po[qs])
                        else:
                            nc.vector.tensor_copy(dst, po[qs])
                    elif copy_split and qs >= 2:
                        nc.scalar.copy(dst, po[qs])
                    else:
                        nc.vector.tensor_copy(dst, po[qs])
                if jb == out_batch - 1 or j == nslot - 1:
                    if not skip_out_dma:
                        j0 = ob_state["j0"]
                        nb = j - j0 + 1
                        out_engine.dma_start(
                            out_d[:, j0 * W : j0 * W + nb * W], ob[:, 0 : nb * W]
                        )

            # Units are processed in pairs sharing one ACT instruction.
            # Software-pipelined emission: pair p+1's S matmuls are emitted
            # before pair p's PV matmuls so the PE never waits on the exp.
            pairs = []
            i = 0
            while i < len(units):
                pairs.append(units[i : i + 2])
                i += 2
            npairs = len(pairs)

            def emit_s(p):
                ps = psum.tile([128, 1024], F32, tag="ps", bufs=ps_bufs, name="ps")
                for pi, (j, t, first, last) in enumerate(pairs[p]):
                    for c in range(NDC):
                        nc.tensor.matmul(
                            ps[:, pi * 512 : (pi + 1) * 512],
                            kt_ap(j, t, c),
                            qt_ap(j, c),
                            start=(c == 0),
                            stop=(c == NDC - 1),
                        )
                return ps

            def emit_exp(p, ps):
                npair = len(pairs[p])
                pt = work.tile(
                    [128, npair * 512], BF16, tag=f"pt{npair}", bufs=pt_bufs, name="pt"
                )
                nc.scalar.activation(pt, ps[:, 0 : npair * 512], EXP, scale=0.0625)
                return pt

            def emit_pv(p, pt):
                nonlocal po
                for pi, (j, t, first, last) in enumerate(pairs[p]):
                    if first:
                        po = open_po()
                    vp_a = vp_ap(j, t)
                    for qs in range(NQS):
                        nc.tensor.matmul(
                            po[qs],
                            pt[:, pi * 512 + qs * 128 : pi * 512 + (qs + 1) * 128],
                            vp_a,
                            start=first,
                            stop=last,
                        )
                    if last:
                        close_po(j)
                        k = ph_of[j][0]
                        if (
                            timing_loop
                            and not skip_loads
                            and not preload
                            and j == phases[k][-1]
                        ):
                            # reload this phase (other buffer) for next pass
                            load_phase(k, (1 - cur_pb[0]) % n_pb)

            # Per-unit pipelined emission: ps rotates over `ps_bufs` single-bank
            # PSUM tiles, the exp ACT is per unit [128,512], and S-matmuls run
            # `depth` units ahead so the ACT is never on the PE critical path.
            def emit_s_u(u, pb=None):
                if pb is not None:
                    cur_pb[0] = pb
                j, t, first, last = units[u]
                ps = psum.tile([128, 512], F32, tag="psu", bufs=ps_bufs, name="psu")
                for c in range(NDC):
                    nc.tensor.matmul(
                        ps,
                        kt_ap(j, t, c),
                        qt_ap(j, c),
                        start=(c == 0),
                        stop=(c == NDC - 1),
                    )
                return ps

            def emit_exp_u(u, ps):
                pt = work.tile([128, 512], BF16, tag="ptu", bufs=pt_bufs, name="ptu")
                nc.scalar.activation(pt, ps, EXP, scale=0.0625)
                return pt

            def emit_pv_u(u, pt, pb=None):
                nonlocal po
                if pb is not None:
                    cur_pb[0] = pb
                j, t, first, last = units[u]
                if first:
                    po = open_po()
                vp_a = vp_ap(j, t)
                for qs in range(NQS):
                    nc.tensor.matmul(
                        po[qs],
                        pt[:, qs * 128 : (qs + 1) * 128],
                        vp_a,
                        start=first,
                        stop=last,
                    )
                if last:
                    close_po(j)
                    k = ph_of[j][0]
                    if (
                        timing_loop
                        and not skip_loads
                        and not preload
                        and j == phases[k][-1]
                    ):
                        load_phase(k, (1 - cur_pb[0]) % n_pb)

            if skip_compute:
                pairs = []
                units_l = []
                if not skip_loads:
                    for k in range(len(phases)):
                        load_phase(k)
            else:
                units_l = units

            def emit_pass():
                if preload and timing_loop and not skip_loads:
                    for k in range(len(phases)):
                        load_phase(k, (1 - cur_pb[0]) % n_pb)
                if unit_act:
                    n = len(units_l)
                    pts = {}
                    for v in range(min(depth, n)):
                        pts[v] = emit_exp_u(v, emit_s_u(v))
                    for u in range(n):
                        if u + depth < n:
                            pts[u + depth] = emit_exp_u(
                                u + depth, emit_s_u(u + depth)
                            )
                        emit_pv_u(u, pts.pop(u))
                else:
                    if pairs and not skip_compute:
                        pt_p = emit_exp(0, emit_s(0))
                    for p in range(npairs):
                        if skip_compute:
                            break
                        ps_n = emit_s(p + 1) if p + 1 < npairs else None
                        emit_pv(p, pt_p)
                        if ps_n is not None:
                            pt_p = emit_exp(p + 1, ps_n)

            if fuse_passes and unit_act and not skip_compute and units_l:
                n = len(units_l)
                stream = [(pb, u) for pb in range(n_pb) for u in range(n)]
                ns = len(stream)
                cap_lead = min(ps_bufs, pt_bufs) - 1
                pts = {}
                nexts = 0

                def emit_ahead(tgt):
                    nonlocal nexts
                    while nexts < ns and nexts <= tgt and nexts - 0 >= 0:
                        pb2, u2 = stream[nexts]
                        if u2 == 0 and preload and timing_loop and not skip_loads:
                            for k in range(len(phases)):
                                load_phase(k, (1 - pb2) % n_pb)
                        pts[nexts] = emit_exp_u(u2, emit_s_u(u2, pb=pb2))
                        nexts += 1

                emit_ahead(depth - 1)
                for i in range(ns):
                    pb, u = stream[i]
                    # at a slot's last unit, pull one extra S group ahead so
                    # the PE has independent work while po is copied out
                    extra = 1 if units[u][3] else 0
                    emit_ahead(min(i + depth + extra, i + cap_lead))
                    emit_pv_u(u, pts.pop(i), pb=pb)
            else:
                for pb in range(n_pb):
                    cur_pb[0] = pb
                    emit_pass()

    nc.compile()
    nc._passes_per_iter = n_pb
    return nc


def get_program(caps, timing_loop=False, **opts):
    key = (tuple(caps), bool(timing_loop), tuple(sorted(opts.items())))
    if key not in _progs:
        _progs[key] = _build_program(tuple(caps), timing_loop=timing_loop, **opts)
    return _progs[key]


def plan(valid_len):
    """Split work into chunks and deal them into a static slot template.

    Returns (caps, assign) where caps[j] = units in slot j (same all cores)
    and assign[c][j] = (b, qi, k0, klen) or None.
    """
    vl = np.asarray(valid_len).astype(np.int64)
    nkb = [max(1, -(-int(v) // KB)) for v in vl]
    chunks = []
    for b in range(B):
        for qi in range(LQ // QT):
            n, k0 = nkb[b], 0
            while n > 0:
                l = min(SMAX, n)
                chunks.append((b, qi, k0, l))
                k0 += l
                n -= l
    chunks.sort(key=lambda ch: -ch[3])
    nslot = -(-len(chunks) // NCORES)
    caps = tuple(chunks[j * NCORES][3] for j in range(nslot))
    assign = [[None] * nslot for _ in range(NCORES)]
    for idx, ch in enumerate(chunks):
        j, c = divmod(idx, NCORES)
        assign[c][j] = ch
    return caps, assign


def pack_core_inputs(query, key, value, valid_len, caps, assign_c):
    """Pack one core's inputs into the flat phase-major layout."""
    bf16 = ml_dtypes.bfloat16
    phases, pbase, pkt_off, pvp_off, ptot = phase_layout(caps)
    vl = np.asarray(valid_len).astype(np.int64)
    flat = np.zeros((128, ptot), dtype=bf16)
    for k, ph in enumerate(phases):
        nph = len(ph)
        pc = sum(caps[j] for j in ph)
        qtS = np.zeros((128, NDC, nph * QT), dtype=bf16)
        ktP = np.zeros((128, NDC, pc * KB), dtype=bf16)
        vpP = np.zeros((128, pc, DV + 1), dtype=bf16)
        u0 = 0
        for joff, j in enumerate(ph):
            ch = assign_c[j]
            cj = caps[j]
            if ch is not None:
                b, qi, k0, klen = ch
                qs = query[b, qi * QT : (qi + 1) * QT, :].T.reshape(NDC, 128, QT)
                qtS[:, :, joff * QT : (joff + 1) * QT] = qs.transpose(1, 0, 2).astype(
                    bf16
                )
                kbeg, kend = k0 * KB, (k0 + klen) * KB
                ks = key[b, kbeg:kend, :].T.reshape(NDC, 128, klen * KB)
                ktP[:, :, u0 * KB : (u0 + klen) * KB] = ks.transpose(1, 0, 2).astype(
                    bf16
                )
                vv = np.concatenate(
                    [value[b, kbeg:kend], np.ones((klen * KB, 1), np.float32)], axis=1
                )
                vv[vl[b] - kbeg :] = 0.0  # mask: zero rows for k >= valid_len
                vpP[:, u0 : u0 + klen, :] = (
                    vv.reshape(klen, KB, DV + 1).transpose(1, 0, 2).astype(bf16)
                )
            u0 += cj
        L = (pbase[k + 1] if k + 1 < len(phases) else ptot) - pbase[k]
        blk = np.concatenate(
            [qtS.reshape(128, -1), ktP.reshape(128, -1), vpP.reshape(128, -1)], axis=1
        )
        assert blk.shape[1] == L
        flat[:, pbase[k] : pbase[k] + L] = blk
    return {"inp": flat}


def kernel(query, key, value, valid_len, _res_out=None):
    query = np.asarray(query, dtype=np.float32)
    key = np.asarray(key, dtype=np.float32)
    value = np.asarray(value, dtype=np.float32)

    caps, assign = plan(valid_len)
    in_maps = [
        pack_core_inputs(query, key, value, valid_len, caps, assign[c])
        for c in range(NCORES)
    ]
    nc = get_program(caps)
    res = run_bass_kernel_spmd(nc, in_maps, core_ids=list(range(NCORES)))
    if _res_out is not None:
        _res_out.append(res)

    nslot = len(caps)
    num = np.zeros((B, LQ // QT, QT, DV), dtype=np.float32)
    den = np.zeros((B, LQ // QT, QT), dtype=np.float32)
    for c in range(NCORES):
        # out layout: [128, nslot*NQS*(DV+1)] -> [128, nslot, NQS, DV+1]
        r = (
            np.asarray(res.results[c]["out"])
            .astype(np.float32)
            .reshape(128, nslot, NQS, DV + 1)
        )
        for j, ch in enumerate(assign[c]):
            if ch is None:
                continue
            b, qi, k0, klen = ch
            blk = r[:, j].transpose(1, 0, 2).reshape(QT, DV + 1)  # q-major
            num[b, qi] += blk[:, :DV]
            den[b, qi] += blk[:, DV]
    out = num / den[..., None]
    return out.reshape(B, LQ, DV).astype(np.float32)



# revision 24
# speedup vs baseline: 1.1717x; 1.0040x over previous
"""Masked dot-product attention (B=16, Lq=Lk=2048, D=Dv=256, fp32) on 8 trn2 cores.

Work-flattened design: the computation is a list of independent units
(batch b, 512-query slot, 128-key block). Units per batch = ceil(valid_len/128)
x 4 query slots; the host splits each (b, q-slot) item into chunks of <= SMAX
key blocks, sorts chunks by size, and deals them round-robin into a static
per-core slot template (caps = per-tier max chunk length, identical on every
core -> one SPMD program, no runtime gating). Pad units have zeroed K/V' so
they contribute nothing. The mask also lives in V' (masked key rows zeroed,
including the appended ones-column), so the exp needs no per-key bias and two
units share one ACT instruction ([128,1024] across 2 PSUM banks).

Per unit: S^T = (K^T blk).T @ (Q^T slot) (2 matmuls N=512, contract d)
          P^T = exp(S^T / 16)             (one ACT per 2 units)
          po[qs] += (P^T slice).T @ V'    (4 matmuls N=257, accumulate over
                                           the chunk; col 256 = sum exp)
Slot end: copy po -> SBUF (bf16) and DMA out; host combines chunk partials
(numerator/denominator sums) and normalizes.
"""

import contextlib

import numpy as np
import ml_dtypes

import concourse.bass as bass
import concourse.bacc as bacc
import concourse.tile as tile
from concourse import mybir
from concourse.bass_utils import run_bass_kernel_spmd

B, LQ, LK, D, DV = 16, 2048, 2048, 256, 256
NCORES = 8
KB = 128               # keys per unit
QT = 512               # queries per slot
NQS = QT // 128        # 4 query sub-tiles per slot
NDC = D // 128         # 2 contraction chunks
SMAX = 8               # max key-blocks per chunk (slot fill)

BF16 = mybir.dt.bfloat16
F32 = mybir.dt.float32
EXP = mybir.ActivationFunctionType.Exp

_progs = {}

PH = 2  # slots per load phase


def phase_layout(caps):
    """Group slots into load phases; flat per-partition element offsets.

    Per phase k (slots ph, pc units): [qt: NDC*nph*QT][kt: NDC*pc*KB][vp: pc*(DV+1)]
    Returns (phases, base[k], kt_off[k], vp_off[k], total).
    """
    nslot = len(caps)
    phases = [list(range(k, min(k + PH, nslot))) for k in range(0, nslot, PH)]
    base, kt_off, vp_off = [], [], []
    tot = 0
    for ph in phases:
        nph = len(ph)
        pc = sum(caps[j] for j in ph)
        base.append(tot)
        kt_off.append(NDC * nph * QT)
        vp_off.append(NDC * nph * QT + NDC * pc * KB)
        tot += NDC * nph * QT + NDC * pc * KB + pc * (DV + 1)
    return phases, base, kt_off, vp_off, tot


def _build_program(
    caps,
    timing_loop=False,
    ps_bufs=4,
    pt_bufs=3,
    ob_bufs=6,
    skip_loads=False,
    skip_out=False,
    out_ring="scalar",
    split_loads=1,
    copy_split=True,
    skip_compute=False,
    load_rings=1,
    unit_act=True,
    depth=2,
    copy_pattern="vvvv",
    decouple_loads=False,
    preload=False,
    skip_out_dma=False,
    out_batch=1,
    fuse_passes=True,
    load_frac=1.0,
    stag=True,
    single_pb=False,
):
    """caps: tuple of per-slot unit capacities (same on all cores)."""
    nslot = len(caps)
    cap = sum(caps)
    nc = bacc.Bacc(
        "TRN2",
        target_bir_lowering=False,
        debug=False,
        num_devices=NCORES,
        enable_asserts=False,
    )
    phases, pbase, pkt_off, pvp_off, ptot = phase_layout(caps)
    inp_d = nc.dram_tensor("inp", [128, ptot], BF16, kind="ExternalInput").ap()
    if timing_loop:
        ri_d = nc.dram_tensor("riter", [1, 1], mybir.dt.int32, kind="ExternalInput").ap()
    out_d = nc.dram_tensor(
        "out", [128, nslot * NQS * (DV + 1)], BF16, kind="ExternalOutput"
    ).ap()

    # flattened unit list: (slot j, t within slot, first, last)
    units = []
    for j, cj in enumerate(caps):
        for t in range(cj):
            units.append((j, t, t == 0, t == cj - 1))

    ubase = [sum(caps[:j]) for j in range(nslot)]

    with tile.TileContext(nc) as tc:
        with (
            tc.tile_pool(name="inp", bufs=1) as inp,
            tc.tile_pool(name="work", bufs=3) as work,
            tc.tile_pool(name="outp", bufs=2) as outp,
            tc.tile_pool(name="psum", bufs=1, space="PSUM") as psum,
            contextlib.ExitStack() as body_cm,
        ):
            if timing_loop:
                ri_sb = work.tile([1, 1], mybir.dt.int32, tag="ri", bufs=1)
                nc.sync.dma_start(ri_sb, ri_d)
                riter = nc.values_load(
                    ri_sb, min_val=1, max_val=1 << 20, skip_runtime_bounds_check=True
                )
            # Preload the exp table set (~2.7us) before the main body.
            warm_in = work.tile([128, 1], F32, tag="warm", bufs=1)
            warm_out = work.tile([128, 1], F32, tag="warm2", bufs=1)
            nc.vector.memset(warm_in, 0.0)
            nc.scalar.activation(warm_out, warm_in, EXP, bias=warm_in, scale=1.0)

            # Input tiles: one flat contiguous [128, L] tile per load phase
            # (PH slots) -> a single max-bandwidth DMA per phase. In the
            # timing loop the body is unrolled into TWO passes with
            # double-buffered phase tiles: pass p computes from buffer p while
            # its per-phase reloads fill buffer 1-p for the next pass, so
            # loads never gate compute.
            ph_of = {}
            ph_pc = []
            for k, ph in enumerate(phases):
                for j in ph:
                    ph_of[j] = (k, ph.index(j), sum(caps[i] for i in ph[: ph.index(j)]))
                ph_pc.append(sum(caps[j] for j in ph))
            # Double-buffering needs 2x the phase bytes in SBUF; fall back to
            # single-buffered (slower, loop-carried load deps) when the draw
            # is too large to fit both buffers (~208KB/partition usable).
            fits_2pb = ptot * 4 + 16 * 1024 <= 206 * 1024
            n_pb = 2 if (timing_loop and not skip_loads and fits_2pb and not single_pb) else 1
            ph_tiles_pb = []
            for pb in range(n_pb):
                tiles = []
                for k, ph in enumerate(phases):
                    L = (pbase[k + 1] if k + 1 < len(phases) else ptot) - pbase[k]
                    tiles.append(
                        inp.tile([128, L], BF16, tag=f"ph{k}p{pb}", name=f"ph{k}p{pb}")
                    )
                ph_tiles_pb.append(tiles)
            ph_tiles = ph_tiles_pb[0]
            cur_pb = [0]
            if skip_loads:
                for k in range(len(phases)):
                    nc.vector.memset(ph_tiles[k], 0.01)

            if decouple_loads:
                dummy = inp.tile([128, 1024], BF16, tag="dummy", bufs=1, name="dummy")
                nc.vector.memset(dummy, 0.01)

                def kt_ap(j, t, c):
                    return dummy[:, 0:KB]

                def qt_ap(j, c):
                    return dummy[:, 0:QT]

                def vp_ap(j, t):
                    return dummy[:, 0 : DV + 1]
            else:

                def kt_ap(j, t, c):
                    k, _, uoff = ph_of[j]
                    off = pkt_off[k] + c * ph_pc[k] * KB + (uoff + t) * KB
                    return ph_tiles_pb[cur_pb[0]][k][:, off : off + KB]

                def qt_ap(j, c):
                    k, joff, _ = ph_of[j]
                    off = c * len(phases[k]) * QT + joff * QT
                    return ph_tiles_pb[cur_pb[0]][k][:, off : off + QT]

                def vp_ap(j, t):
                    k, _, uoff = ph_of[j]
                    off = pvp_off[k] + (uoff + t) * (DV + 1)
                    return ph_tiles_pb[cur_pb[0]][k][:, off : off + (DV + 1)]

            def load_phase(k, pb=0):
                L = (pbase[k + 1] if k + 1 < len(phases) else ptot) - pbase[k]
                Lf = max(1, int(L * load_frac))
                if load_rings == 1:
                    eng = nc.sync
                elif load_rings == 2:
                    eng = nc.sync if k % 2 == 0 else nc.scalar
                elif load_rings == 3:
                    # split by destination pass-buffer
                    eng = nc.sync if pb == 0 else nc.scalar
                else:
                    eng = nc.gpsimd
                eng.dma_start(
                    ph_tiles_pb[pb][k][:, 0:Lf], inp_d[:, pbase[k] : pbase[k] + Lf]
                )

            if not skip_loads:
                # Prologue load of every phase. In the timing loop, each
                # phase's load is RE-issued inside the body right after that
                # phase's compute (data for the next iteration) so DMA spreads
                # across the whole iteration and never stalls compute.
                for k in range(len(phases)):
                    load_phase(k)

            if timing_loop:
                body_cm.enter_context(
                    tc.For_i(0, riter, staggered_reset=stag)
                )

            po = None
            out_engine = {"scalar": nc.scalar, "sync": nc.sync, "gpsimd": nc.gpsimd}[
                out_ring
            ]

            def open_po():
                return [
                    psum.tile([128, DV + 1], F32, tag=f"po{qs}", bufs=1, name=f"po{qs}")
                    for qs in range(NQS)
                ]

            ob_state = {"ob": None, "j0": -1}

            def close_po(j):
                if skip_out:
                    return
                W = NQS * (DV + 1)
                jb = j % out_batch
                if jb == 0:
                    ob_state["ob"] = outp.tile(
                        [128, W * out_batch], BF16, tag="ob", bufs=ob_bufs, name="ob"
                    )
                    ob_state["j0"] = j
                ob = ob_state["ob"]
                for qs in range(NQS):
                    dst = ob[:, jb * W + qs * (DV + 1) : jb * W + (qs + 1) * (DV + 1)]
                    if unit_act:
                        if copy_pattern[qs] == "s":
                            nc.scalar.copy(dst, po[qs])
                        else:
                            nc.vector.tensor_copy(dst, po[qs])
                    elif copy_split and qs >= 2:
                        nc.scalar.copy(dst, po[qs])
                    else:
                        nc.vector.tensor_copy(dst, po[qs])
                if jb == out_batch - 1 or j == nslot - 1:
                    if not skip_out_dma:
                        j0 = ob_state["j0"]
                        nb = j - j0 + 1
                        out_engine.dma_start(
                            out_d[:, j0 * W : j0 * W + nb * W], ob[:, 0 : nb * W]
                        )

            # Units are processed in pairs sharing one ACT instruction.
            # Software-pipelined emission: pair p+1's S matmuls are emitted
            # before pair p's PV matmuls so the PE never waits on the exp.
            pairs = []
            i = 0
            while i < len(units):
                pairs.append(units[i : i + 2])
                i += 2
            npairs = len(pairs)

            def emit_s(p):
                ps = psum.tile([128, 1024], F32, tag="ps", bufs=ps_bufs, name="ps")
                for pi, (j, t, first, last) in enumerate(pairs[p]):
                    for c in range(NDC):
                        nc.tensor.matmul(
                            ps[:, pi * 512 : (pi + 1) * 512],
                            kt_ap(j, t, c),
                            qt_ap(j, c),
                            start=(c == 0),
                            stop=(c == NDC - 1),
                        )
                return ps

            def emit_exp(p, ps):
                npair = len(pairs[p])
                pt = work.tile(
                    [128, npair * 512], BF16, tag=f"pt{npair}", bufs=pt_bufs, name="pt"
                )
                nc.scalar.activation(pt, ps[:, 0 : npair * 512], EXP, scale=0.0625)
                return pt

            def emit_pv(p, pt):
                nonlocal po
                for pi, (j, t, first, last) in enumerate(pairs[p]):
                    if first:
                        po = open_po()
                    vp_a = vp_ap(j, t)
                    for qs in range(NQS):
                        nc.tensor.matmul(
                            po[qs],
                            pt[:, pi * 512 + qs * 128 : pi * 512 + (qs + 1) * 128],
                            vp_a,
                            start=first,
                            stop=last,
                        )
                    if last:
                        close_po(j)
                        k = ph_of[j][0]
                        if (
                            timing_loop
                            and not skip_loads
                            and not preload
                            and j == phases[k][-1]
                        ):
                            # reload this phase (other buffer) for next pass
                            load_phase(k, (1 - cur_pb[0]) % n_pb)

            # Per-unit pipelined emission: ps rotates over `ps_bufs` single-bank
            # PSUM tiles, the exp ACT is per unit [128,512], and S-matmuls run
            # `depth` units ahead so the ACT is never on the PE critical path.
            def emit_s_u(u, pb=None):
                if pb is not None:
                    cur_pb[0] = pb
                j, t, first, last = units[u]
                ps = psum.tile([128, 512], F32, tag="psu", bufs=ps_bufs, name="psu")
                for c in range(NDC):
                    nc.tensor.matmul(
                        ps,
                        kt_ap(j, t, c),
                        qt_ap(j, c),
                        start=(c == 0),
                        stop=(c == NDC - 1),
                    )
                return ps

            def emit_exp_u(u, ps):
                pt = work.tile([128, 512], BF16, tag="ptu", bufs=pt_bufs, name="ptu")
                nc.scalar.activation(pt, ps, EXP, scale=0.0625)
                return pt

            def emit_pv_u(u, pt, pb=None):
                nonlocal po
                if pb is not None:
                    cur_pb[0] = pb
                j, t, first, last = units[u]
                if first:
                    po = open_po()
                vp_a = vp_ap(j, t)
                for qs in range(NQS):
                    nc.tensor.matmul(
                        po[qs],
                        pt[:, qs * 128 : (qs + 1) * 128],
                        vp_a,
                        start=first,
                        stop=last,
                    )
                if last:
                    close_po(j)
                    k = ph_of[j][0]
                    if (
                        timing_loop
                        and not skip_loads
                        and not preload
                        and j == phases[k][-1]
                    ):
                        load_phase(k, (1 - cur_pb[0]) % n_pb)

            if skip_compute:
                pairs = []
                units_l = []
                if not skip_loads:
                    for k in range(len(phases)):
                        load_phase(k)
            else:
                units_l = units

            def emit_pass():
                if preload and timing_loop and not skip_loads:
                    for k in range(len(phases)):
                        load_phase(k, (1 - cur_pb[0]) % n_pb)
                if unit_act:
                    n = len(units_l)
                    pts = {}
                    for v in range(min(depth, n)):
                        pts[v] = emit_exp_u(v, emit_s_u(v))
                    for u in range(n):
                        if u + depth < n:
                            pts[u + depth] = emit_exp_u(
                                u + depth, emit_s_u(u + depth)
                            )
                        emit_pv_u(u, pts.pop(u))
                else:
                    if pairs and not skip_compute:
                        pt_p = emit_exp(0, emit_s(0))
                    for p in range(npairs):
                        if skip_compute:
                            break
                        ps_n = emit_s(p + 1) if p + 1 < npairs else None
                        emit_pv(p, pt_p)
                        if ps_n is not None:
                            pt_p = emit_exp(p + 1, ps_n)

            if fuse_passes and unit_act and not skip_compute and units_l:
                n = len(units_l)
                stream = [(pb, u) for pb in range(n_pb) for u in range(n)]
                ns = len(stream)
                cap_lead = min(ps_bufs, pt_bufs) - 1
                pts = {}
                nexts = 0

                def emit_ahead(tgt):
                    nonlocal nexts
                    while nexts < ns and nexts <= tgt and nexts - 0 >= 0:
                        pb2, u2 = stream[nexts]
                        if u2 == 0 and preload and timing_loop and not skip_loads:
                            for k in range(len(phases)):
                                load_phase(k, (1 - pb2) % n_pb)
                        pts[nexts] = emit_exp_u(u2, emit_s_u(u2, pb=pb2))
                        nexts += 1

                emit_ahead(depth - 1)
                for i in range(ns):
                    pb, u = stream[i]
                    # at a slot's last unit, pull one extra S group ahead so
                    # the PE has independent work while po is copied out
                    extra = 1 if units[u][3] else 0
                    emit_ahead(min(i + depth + extra, i + cap_lead))
                    emit_pv_u(u, pts.pop(i), pb=pb)
            else:
                for pb in range(n_pb):
                    cur_pb[0] = pb
                    emit_pass()

    nc.compile()
    nc._passes_per_iter = n_pb
    return nc


def get_program(caps, timing_loop=False, **opts):
    key = (tuple(caps), bool(timing_loop), tuple(sorted(opts.items())))
    if key not in _progs:
        try:
            _progs[key] = _build_program(tuple(caps), timing_loop=timing_loop, **opts)
        except Exception:
            if timing_loop and not opts.get("single_pb"):
                # SBUF pressure from double-buffered phases: retry single-buffered
                _progs[key] = _build_program(
                    tuple(caps), timing_loop=timing_loop, single_pb=True, **opts
                )
            else:
                raise
    return _progs[key]


def plan(valid_len):
    """Split work into chunks and deal them into a static slot template.

    Returns (caps, assign) where caps[j] = units in slot j (same all cores)
    and assign[c][j] = (b, qi, k0, klen) or None.
    """
    vl = np.asarray(valid_len).astype(np.int64)
    nkb = [max(1, -(-int(v) // KB)) for v in vl]
    chunks = []
    for b in range(B):
        for qi in range(LQ // QT):
            n, k0 = nkb[b], 0
            while n > 0:
                l = min(SMAX, n)
                chunks.append((b, qi, k0, l))
                k0 += l
                n -= l
    chunks.sort(key=lambda ch: -ch[3])
    nslot = -(-len(chunks) // NCORES)
    caps = tuple(chunks[j * NCORES][3] for j in range(nslot))
    assign = [[None] * nslot for _ in range(NCORES)]
    for idx, ch in enumerate(chunks):
        j, c = divmod(idx, NCORES)
        assign[c][j] = ch
    return caps, assign


def pack_core_inputs(query, key, value, valid_len, caps, assign_c):
    """Pack one core's inputs into the flat phase-major layout."""
    bf16 = ml_dtypes.bfloat16
    phases, pbase, pkt_off, pvp_off, ptot = phase_layout(caps)
    vl = np.asarray(valid_len).astype(np.int64)
    flat = np.zeros((128, ptot), dtype=bf16)
    for k, ph in enumerate(phases):
        nph = len(ph)
        pc = sum(caps[j] for j in ph)
        qtS = np.zeros((128, NDC, nph * QT), dtype=bf16)
        ktP = np.zeros((128, NDC, pc * KB), dtype=bf16)
        vpP = np.zeros((128, pc, DV + 1), dtype=bf16)
        u0 = 0
        for joff, j in enumerate(ph):
            ch = assign_c[j]
            cj = caps[j]
            if ch is not None:
                b, qi, k0, klen = ch
                qs = query[b, qi * QT : (qi + 1) * QT, :].T.reshape(NDC, 128, QT)
                qtS[:, :, joff * QT : (joff + 1) * QT] = qs.transpose(1, 0, 2).astype(
                    bf16
                )
                kbeg, kend = k0 * KB, (k0 + klen) * KB
                ks = key[b, kbeg:kend, :].T.reshape(NDC, 128, klen * KB)
                ktP[:, :, u0 * KB : (u0 + klen) * KB] = ks.transpose(1, 0, 2).astype(
                    bf16
                )
                vv = np.concatenate(
                    [value[b, kbeg:kend], np.ones((klen * KB, 1), np.float32)], axis=1
                )
                vv[vl[b] - kbeg :] = 0.0  # mask: zero rows for k >= valid_len
                vpP[:, u0 : u0 + klen, :] = (
                    vv.reshape(klen, KB, DV + 1).transpose(1, 0, 2).astype(bf16)
                )
            u0 += cj
        L = (pbase[k + 1] if k + 1 < len(phases) else ptot) - pbase[k]
        blk = np.concatenate(
            [qtS.reshape(128, -1), ktP.reshape(128, -1), vpP.reshape(128, -1)], axis=1
        )
        assert blk.shape[1] == L
        flat[:, pbase[k] : pbase[k] + L] = blk
    return {"inp": flat}


def kernel(query, key, value, valid_len, _res_out=None):
    query = np.asarray(query, dtype=np.float32)
    key = np.asarray(key, dtype=np.float32)
    value = np.asarray(value, dtype=np.float32)

    caps, assign = plan(valid_len)
    in_maps = [
        pack_core_inputs(query, key, value, valid_len, caps, assign[c])
        for c in range(NCORES)
    ]
    nc = get_program(caps)
    res = run_bass_kernel_spmd(nc, in_maps, core_ids=list(range(NCORES)))
    if _res_out is not None:
        _res_out.append(res)

    nslot = len(caps)
    num = np.zeros((B, LQ // QT, QT, DV), dtype=np.float32)
    den = np.zeros((B, LQ // QT, QT), dtype=np.float32)
    for c in range(NCORES):
        # out layout: [128, nslot*NQS*(DV+1)] -> [128, nslot, NQS, DV+1]
        r = (
            np.asarray(res.results[c]["out"])
            .astype(np.float32)
            .reshape(128, nslot, NQS, DV + 1)
        )
        for j, ch in enumerate(assign[c]):
            if ch is None:
                continue
            b, qi, k0, klen = ch
            blk = r[:, j].transpose(1, 0, 2).reshape(QT, DV + 1)  # q-major
            num[b, qi] += blk[:, :DV]
            den[b, qi] += blk[:, DV]
    out = num / den[..., None]
    return out.reshape(B, LQ, DV).astype(np.float32)

